# revision 54
# baseline (speedup 1.0000x reference)
"""Trainium2 Bass kernel for packed-sequence GQA attention (nn_Attention_84602265796942).

Sharding: data-parallel over the B=16 packed sequences -> 2 sequences (1024
tokens) per core, weights replicated. Zero collectives.

Per-core pipeline (all matmuls bf16 x bf16 -> fp32 PSUM):
  A0) One fp32 warm-up matmul on junk data carries the PE through its
      p-state frequency ramp while the first input DMAs are in flight; then
      the V projection, k-major across 6 concurrent PSUM chains so the PE
      starts as soon as the first xT/wvT tiles land (hides the input DMA);
      then the K projection + RoPE. RoPE is applied via a host-side head-dim
      permutation ([a0..a15 b0..b15 a16..a31 b16..b31]) so the rotation
      partner sits at partition r^16 (one stream_shuffle).
  A1) Q projection + attention per (block, head-pair), pipelined by the tile
      scheduler: q-heads are host-permuted so pair (h, h+4) shares a qt tile
      and maps to kv heads (2g, 2g+1) = the two partition halves of one K
      tile; scores are computed transposed (scoresT[m,l]); softmax without
      max subtraction (scores are bounded). PV runs at full PE utilization
      with probsT as the stationary operand, producing o[l, d] tiles
      (out free size 64 at 128 partitions instead of free 512 at 65);
      softmax denominators come from near-free 1-column matmuls against the
      ones column of the V tiles; normalization is one broadcast (stride-0)
      tensor multiply with per-partition reciprocals, emitted after ALL
      eight PV chains so the chains are not serialized against the
      normalize reads (matmul start=True counts as a whole-tile write);
      the normalized o[l, d] tiles are transposed back to attT[d, l] with
      PE transpose-mode matmuls. The unit epilogue (normalize/transpose) is
      emitted after the NEXT unit's Q chain so the PE always has
      independent work while the DVE runs.
  C)  Output projection from attT. wo is pre-split into halves; the first
      half is DMA-prefetched during attention, and the first four output
      chains are emitted inside A1 out of the freed psQ banks — filler work
      for the PE while the last unit's scores wait on the exp cadence.
      Outputs are staged in [128, 1024] tiles to halve the output DMA
      count; the final chain/store is split small so the end-of-kernel
      drain is short.

DMAs are batched (one descriptor block per weight slab / wo half) to keep
the flat per-DMA DGE overhead off the critical path.
"""
import numpy as np
import ml_dtypes

import concourse.bass as bass
import concourse.tile as tile
from concourse import bacc, masks, mybir
from concourse.bass import broadcast_tensor_aps
from concourse.bass_utils import run_bass_kernel_spmd

F32 = mybir.dt.float32
BF16 = mybir.dt.bfloat16

B, L, DIM, H, HKV, DH = 16, 512, 2048, 32, 8, 64
REP = H // HKV
S = B * L
NCORE = 8
S_LOC = S // NCORE          # 1024 tokens per core
NBLK = S_LOC // L           # 2 blocks per core
SCALE = DH ** -0.5

# within-head dim permutation: rows [a0..a15, b0..b15, a16..a31, b16..b31]
PERM64 = np.concatenate([np.arange(0, 32, 2), np.arange(1, 32, 2),
                         np.arange(32, 64, 2), np.arange(33, 64, 2)])
_rr = np.arange(64)
FREQ_IDX = (_rr // 32) * 16 + (_rr % 16)
C2_SIGN = np.where((_rr % 32) < 16, -1.0, 1.0).astype(np.float32)
# q-head order: pair (h, h+4) within each group of 8 -> kv heads (2g, 2g+1)
HPERM = np.array([8 * gi + t + 4 * half
                  for gi in range(4) for t in range(4) for half in range(2)])

_CACHED = {}

LAST_RESULTS = None  # BassKernelResults of the most recent run (for test.py)


def _build():
    nc = bacc.Bacc("TRN2", target_bir_lowering=False, debug=False,
                   num_devices=NCORE)

    KD = DIM // 128          # 16 contraction tiles
    NQI = (H * DH) // 128    # 16 Q row-tiles (one head pair each)
    NKI = (HKV * DH) // 128  # 4 K row-tiles
    NMT = L // 128           # 4 token tiles per block
    NM = S_LOC // 128        # 8 token tiles per core
    EXP = mybir.ActivationFunctionType.Exp
    SHUF_MASK = [i ^ 16 for i in range(32)]

    xT_d = nc.dram_tensor("xT", [128, KD, S_LOC], BF16, kind="ExternalInput")
    wq_d = nc.dram_tensor("wqT", [128, 4, KD, 512], BF16, kind="ExternalInput")
    wk_d = nc.dram_tensor("wkT", [128, KD, HKV * DH], BF16, kind="ExternalInput")
    wv_d = nc.dram_tensor("wvT", [128, KD, HKV * DH], BF16, kind="ExternalInput")
    wo_d = nc.dram_tensor("woT", [128, 2, NQI, DIM // 2], BF16, kind="ExternalInput")
    c1_d = nc.dram_tensor("c1", [128, S_LOC], BF16, kind="ExternalInput")
    c2_d = nc.dram_tensor("c2", [128, S_LOC], BF16, kind="ExternalInput")
    out_d = nc.dram_tensor("out", [S_LOC, DIM], F32, kind="ExternalOutput")

    with tile.TileContext(nc) as tc:
        with (
            tc.tile_pool(name="persist", bufs=1) as pp,      # long-lived activations
            tc.tile_pool(name="scratch", bufs=2) as sp,      # rope/norm scratch
            tc.tile_pool(name="wo", bufs=1) as wop,          # wo prefetch (A1+C)
        ):
            # persistent activation tensors
            kt = [[pp.tile([128, L], BF16, tag=f"kt{g}_{b}", name=f"kt{g}_{b}")
                   for b in range(NBLK)] for g in range(NKI)]
            vaug = [pp.tile([128, HKV * (DH + 1)], BF16, tag=f"va{m}", name=f"va{m}")
                    for m in range(NM)]
            att = [[pp.tile([128, L], BF16, tag=f"at{i}_{b}", name=f"at{i}_{b}")
                    for b in range(NBLK)] for i in range(NQI)]
            ident = pp.tile([128, 128], BF16, tag="ident", name="ident")
            masks.make_identity(nc, ident[:])

            woL = wop.tile([128, NQI, DIM // 2], BF16, tag="woL", name="woL")

            # ======== phases A0 + A1 (everything that needs x/wq/probs) ========
            with (
                tc.tile_pool(name="wslab", bufs=2) as wsp,   # wq slab double-buffer
                tc.tile_pool(name="qtp", bufs=2) as qtp,     # rotating qt tiles
                tc.tile_pool(name="probs", bufs=5) as probp,
                tc.tile_pool(name="inX", bufs=1) as px,      # xT + rope tables
                # psQ wraps A0+A1 so the first Q chains overlap the A0 tail
                tc.tile_pool(name="psQ", bufs=2, space="PSUM") as psQ,
            ):
                xT = [px.tile([128, S_LOC], BF16, tag=f"xT{k}", name=f"xT{k}")
                      for k in range(KD)]
                c1 = px.tile([128, S_LOC], BF16, tag="c1", name="c1s")
                c2 = px.tile([128, S_LOC], BF16, tag="c2", name="c2s")

                def rope_epilogue(ps, b, dst128):
                    """ps: [128, 512] psum of pre-rope QT/KT rows -> bf16 dst."""
                    cs = slice(b * L, (b + 1) * L)
                    sh = sp.tile([128, L], F32, tag="sh", name="sh")
                    nc.vector.stream_shuffle(sh[:], ps[:], SHUF_MASK)
                    t1 = sp.tile([128, L], BF16, tag="t1", name="t1")
                    nc.vector.tensor_mul(t1[:], ps[:], c1[:, cs])
                    t2 = sp.tile([128, L], BF16, tag="t2", name="t2")
                    nc.vector.tensor_mul(t2[:], sh[:], c2[:, cs])
                    nc.vector.tensor_add(dst128[:], t1[:], t2[:])

                # ================= phase A0: V + K projections =================
                with (
                    tc.tile_pool(name="inW", bufs=1) as pw,
                    tc.tile_pool(name="psH", bufs=6, space="PSUM") as pH,
                ):
                    # p-state warm-up: one fp32 matmul on junk data keeps the
                    # PE busy through its frequency ramp while the first input
                    # DMAs are still in flight, so real matmuls start at full
                    # speed
                    junk = sp.tile([128, L], F32, tag="sh", name="junk")
                    nc.vector.memset(junk[:], 0.5)
                    wps = psQ.tile([128, L], F32, tag="q", name="wps")
                    nc.tensor.matmul(wps[:], junk[:, 0:128], junk[:],
                                     start=True, stop=True)

                    wvT, wkT = [], []
                    for k in range(KD):
                        t = pw.tile([128, HKV * DH], BF16, tag=f"wvT{k}",
                                    name=f"wvT{k}")
                        wvT.append(t)
                        nc.sync.dma_start(xT[k][:], xT_d[:, k, :])
                        nc.sync.dma_start(t[:], wv_d[:, k, :])
                    nc.sync.dma_start(c1[:], c1_d[:])
                    nc.sync.dma_start(c2[:], c2_d[:])
                    for k in range(KD):
                        t = pw.tile([128, HKV * DH], BF16, tag=f"wkT{k}",
                                    name=f"wkT{k}")
                        wkT.append(t)
                        nc.sync.dma_start(t[:], wk_d[:, k, :])
                    slab = wsp.tile([128, KD, 512], BF16, tag="slab", name="slab0")
                    nc.sync.dma_start(slab[:], wq_d[:, 0])

                    for m in range(NM):
                        nc.vector.memset(vaug[m][:], 1.0)

                    def stage_v(m, ps):
                        nc.vector.tensor_copy(
                            vaug[m].rearrange("p (g d) -> p g d",
                                              d=DH + 1)[:, :, 0:DH],
                            ps.rearrange("p (g d) -> p g d", d=DH))

                    # ---- V projection: 6 k-major chains, then 2 m-major ----
                    vps = [pH.tile([128, HKV * DH], F32, tag="h", name="vps")
                           for _ in range(6)]
                    for k in range(KD):
                        for m in range(6):
                            nc.tensor.matmul(
                                vps[m][:], xT[k][:, m * 128:(m + 1) * 128],
                                wvT[k][:],
                                start=(k == 0), stop=(k == KD - 1))
                    for m in range(6):
                        stage_v(m, vps[m])
                    for m in range(6, NM):
                        ps = pH.tile([128, HKV * DH], F32, tag="h", name="vps2")
                        for k in range(KD):
                            nc.tensor.matmul(
                                ps[:], xT[k][:, m * 128:(m + 1) * 128], wvT[k][:],
                                start=(k == 0), stop=(k == KD - 1))
                        stage_v(m, ps)

                    # ---- K projection + rope ----
                    for i in range(NKI):
                        for b in range(NBLK):
                            ps = pH.tile([128, L], F32, tag="h", name="kps")
                            for k in range(KD):
                                nc.tensor.matmul(
                                    ps[:], wkT[k][:, i * 128:(i + 1) * 128],
                                    xT[k][:, b * L:(b + 1) * L],
                                    start=(k == 0), stop=(k == KD - 1))
                            rope_epilogue(ps, b, kt[i][b])

                # ---------- phase A1: Q projection + attention ----------
                with (
                    tc.tile_pool(name="psS", bufs=2, space="PSUM") as psS,
                    tc.tile_pool(name="psOd", bufs=2, space="PSUM") as psOd,
                    tc.tile_pool(name="psOn", bufs=1, space="PSUM") as psOn,
                    tc.tile_pool(name="psT", bufs=1, space="PSUM") as psT,
                ):
                    def attention_body(hp, b, qt_t):
                        """scores/softmax/PV chains for head pair hp, block b."""
                        gi = hp // 4
                        probs = [[], []]      # [half][mi]
                        for mi in range(NMT):
                            se = psS.tile([128, L], F32, tag="s", name="sps")
                            nc.tensor.matmul(
                                se[:],
                                kt[gi][b][0:64, mi * 128:(mi + 1) * 128],
                                qt_t[0:64, :])
                            so = psS.tile([128, L], F32, tag="s", name="sps")
                            nc.tensor.matmul(
                                so[:],
                                kt[gi][b][64:128, mi * 128:(mi + 1) * 128],
                                qt_t[64:128, :])
                            pe = probp.tile([128, L], BF16, tag="pe", name="pe")
                            nc.scalar.activation(pe[:], se[:], EXP, scale=SCALE)
                            po = probp.tile([128, L], BF16, tag="po", name="po")
                            nc.scalar.activation(po[:], so[:], EXP, scale=SCALE)
                            probs[0].append(pe)
                            probs[1].append(po)

                        # softmax denominators: 1-column matmuls vs the ones
                        # column of vaug; all 8 (l-tile, head) chains in one
                        # PSUM tile (col = li*2+he, matching the dat blocks)
                        den = psOn.tile([128, 8], F32, tag="n", name="den")
                        for he in range(2):
                            oc = (2 * gi + he) * (DH + 1) + DH
                            for li in range(NMT):
                                c = li * 2 + he
                                for mi in range(NMT):
                                    nc.tensor.matmul(
                                        den[:, c:c + 1],
                                        probs[he][mi][:, li * 128:(li + 1) * 128],
                                        vaug[b * NMT + mi][:, oc:oc + 1],
                                        start=(mi == 0), stop=(mi == NMT - 1))
                        rd = sp.tile([128, 8], F32, tag="rd", name="rd")
                        nc.vector.reciprocal(rd[:], den[:])

                        # PV at full PE utilization: o[l, d] with probsT
                        # stationary
                        dat = psOd.tile([128, L], F32, tag="d", name="dat")
                        for li in range(NMT):
                            for he in range(2):
                                g = 2 * gi + he
                                c0 = li * 128 + he * 64
                                for mi in range(NMT):
                                    nc.tensor.matmul(
                                        dat[:, c0:c0 + 64],
                                        probs[he][mi][:, li * 128:(li + 1) * 128],
                                        vaug[b * NMT + mi][:, g * (DH + 1):
                                                           g * (DH + 1) + DH],
                                        start=(mi == 0), stop=(mi == NMT - 1))
                        return dat, rd

                    def attention_finish(hp, b, dat, rd):
                        """normalize + transpose back to attT[d, l]; emitted
                        after the NEXT unit's Q chain so the PE always has
                        independent work while the DVE normalize runs."""
                        # single broadcast multiply: per-partition scalars per
                        # 64-column block
                        attn = sp.tile([128, L], BF16, tag="attn", name="attn")
                        datv = dat.rearrange("p (c d) -> p c d", d=DH)
                        attnv = attn.rearrange("p (c d) -> p c d", d=DH)
                        rdv = rd.rearrange("p (c o) -> p c o", o=1)
                        rdb, _ = broadcast_tensor_aps(rdv, datv)
                        nc.vector.tensor_mul(attnv, datv, rdb)

                        tp = psT.tile([128, L], BF16, tag="t", name="tp")
                        for li in range(NMT):
                            nc.tensor.transpose(
                                tp[:, li * 128:(li + 1) * 128],
                                attn[:, li * 128:(li + 1) * 128], ident[:])
                        nc.vector.tensor_copy(att[hp][b][:], tp[:])

                    cur = slab
                    fin = None        # (hp, b, dat, rd) awaiting finish
                    for ig in range(4):
                        if ig < 3:
                            nxt = wsp.tile([128, KD, 512], BF16, tag="slab",
                                           name=f"slab{ig + 1}")
                            nc.sync.dma_start(nxt[:], wq_d[:, ig + 1])
                        if ig == 2:
                            nc.sync.dma_start(woL[:], wo_d[:, 0])
                        for ii in range(4):
                            i = ig * 4 + ii
                            for b in range(NBLK):
                                ps = psQ.tile([128, L], F32, tag="q", name="qps")
                                for k in range(KD):
                                    nc.tensor.matmul(
                                        ps[:], cur[:, k, ii * 128:(ii + 1) * 128],
                                        xT[k][:, b * L:(b + 1) * L],
                                        start=(k == 0), stop=(k == KD - 1))
                                qt_t = qtp.tile([128, L], BF16, tag="qt",
                                                name=f"qt{i}_{b}")
                                rope_epilogue(ps, b, qt_t)
                                if fin is not None:
                                    attention_finish(*fin)
                                dat, rd = attention_body(i, b, qt_t)
                                fin = (i, b, dat, rd)
                        if ig < 3:
                            cur = nxt

                    # early output-projection chains (b=0, left half): they
                    # only need att[*][0] (complete after unit 30) and the
                    # prefetched woL, and they run out of the freed psQ banks
                    # — filler work for the PE while the last unit's scores
                    # wait on the exp cadence
                    for st in range(NMT):
                        ot = sp.tile([128, DIM // 2], F32, tag="ot", name="ot_e",
                                     bufs=1)
                        for e2 in range(2):
                            ps = psQ.tile([128, 512], F32, tag="q", name="qps")
                            for kq in range(NQI):
                                nc.tensor.matmul(
                                    ps[:],
                                    att[kq][0][:, st * 128:(st + 1) * 128],
                                    woL[:, kq, e2 * 512:(e2 + 1) * 512],
                                    start=(kq == 0), stop=(kq == NQI - 1))
                            nc.vector.tensor_copy(
                                ot[:, e2 * 512:(e2 + 1) * 512], ps[:])
                        nc.sync.dma_start(
                            out_d[st * 128:(st + 1) * 128, 0:1024], ot[:])
                    attention_finish(*fin)

            # ================= phase C: output projection =================
            with (
                tc.tile_pool(name="outC", bufs=1) as pc,
                tc.tile_pool(name="outsb", bufs=4) as op,
                tc.tile_pool(name="psC", bufs=4, space="PSUM") as psC,
            ):
                woR = pc.tile([128, NQI, DIM // 2], BF16, tag="woR", name="woR")
                nc.sync.dma_start(woR[:], wo_d[:, 1])
                # (b=0, half=0) already done inside A1; order the rest so the
                # woR-dependent groups come after a woL-fed group
                for b, half in ((1, 0), (0, 1), (1, 1)):
                    wot = woL if half == 0 else woR
                    for st in range(NMT):           # token tile within block
                        last = (half == 1 and b == NBLK - 1 and st == NMT - 1)
                        ot = op.tile([128, DIM // 2], F32, tag="ot", name="ot")
                        r0 = b * L + st * 128
                        # the very last chain/store is split small so the
                        # end-of-kernel copy+DMA drain is short
                        splits = ((0, 512), (512, 1024)) if not last else \
                                 ((0, 512), (512, 896), (896, 1024))
                        for (c0, c1_) in splits:
                            ps = psC.tile([128, 512], F32, tag="c", name="cps")
                            for kq in range(NQI):
                                nc.tensor.matmul(
                                    ps[:, 0:c1_ - c0],
                                    att[kq][b][:, st * 128:(st + 1) * 128],
                                    wot[:, kq, c0:c1_],
                                    start=(kq == 0), stop=(kq == NQI - 1))
                            nc.vector.tensor_copy(
                                ot[:, c0:c1_], ps[:, 0:c1_ - c0])
                            if last:
                                nc.sync.dma_start(
                                    out_d[r0:r0 + 128,
                                          half * 1024 + c0:half * 1024 + c1_],
                                    ot[:, c0:c1_])
                        if not last:
                            nc.sync.dma_start(
                                out_d[r0:r0 + 128,
                                      half * 1024:(half + 1) * 1024],
                                ot[:])

    nc.compile()
    return nc


def _prep_shared(wq, wk, wv, wo):
    bf = ml_dtypes.bfloat16
    KD = DIM // 128

    # wq: head order HPERM, PERM64 within head
    wq_p = wq.reshape(H, DH, DIM)[HPERM][:, PERM64, :].reshape(H * DH, DIM)
    # wk: natural head order, PERM64 within head
    wk_p = wk.reshape(HKV, DH, DIM)[:, PERM64, :].reshape(HKV * DH, DIM)
    # wo columns: head order HPERM, dims unpermuted (V is not roped)
    wo_p = wo.reshape(DIM, H, DH)[:, HPERM, :].reshape(DIM, H * DH)

    # transposed + retiled for batched DMA: leading dim = SBUF partition
    wqT = np.ascontiguousarray(
        wq_p.T.reshape(KD, 128, 4, 512).transpose(1, 2, 0, 3).astype(bf))
    wkT = np.ascontiguousarray(
        wk_p.T.reshape(KD, 128, HKV * DH).transpose(1, 0, 2).astype(bf))
    wvT = np.ascontiguousarray(
        wv.T.reshape(KD, 128, HKV * DH).transpose(1, 0, 2).astype(bf))
    woT = np.ascontiguousarray(
        wo_p.T.reshape(KD, 128, 2, DIM // 2).transpose(1, 2, 0, 3).astype(bf))
    return wqT, wkT, wvT, woT


def kernel(x, freqs_cos, freqs_sin, wq, wk, wv, wo):
    global LAST_RESULTS
    x = np.asarray(x, np.float32)
    freqs_cos = np.asarray(freqs_cos, np.float32)
    freqs_sin = np.asarray(freqs_sin, np.float32)
    bf = ml_dtypes.bfloat16
    KD = DIM // 128

    if "nc" not in _CACHED:
        _CACHED["nc"] = _build()
    nc = _CACHED["nc"]

    wqT, wkT, wvT, woT = _prep_shared(
        np.asarray(wq, np.float32), np.asarray(wk, np.float32),
        np.asarray(wv, np.float32), np.asarray(wo, np.float32))

    in_maps = []
    for c in range(NCORE):
        rows = slice(c * S_LOC, (c + 1) * S_LOC)
        xT = np.ascontiguousarray(
            x[rows].T.reshape(KD, 128, S_LOC).transpose(1, 0, 2).astype(bf))
        fcc = freqs_cos[rows]      # [S_LOC, 32]
        fss = freqs_sin[rows]
        c1h = fcc[:, FREQ_IDX].T   # [64, S_LOC]
        c2h = (fss[:, FREQ_IDX] * C2_SIGN[None, :]).T
        c1 = np.ascontiguousarray(np.concatenate([c1h, c1h], 0).astype(bf))
        c2 = np.ascontiguousarray(np.concatenate([c2h, c2h], 0).astype(bf))
        in_maps.append({"xT": xT, "wqT": wqT, "wkT": wkT, "wvT": wvT,
                        "woT": woT, "c1": c1, "c2": c2})

    res = None
    for attempt in range(3):
        try:
            res = run_bass_kernel_spmd(nc, in_maps, list(range(NCORE)))
            break
        except Exception:
            if attempt == 2:
                raise
            import time
            time.sleep(10)   # transient NRT device errors usually clear on retry
    LAST_RESULTS = res
    out = np.concatenate([res.results[c]["out"] for c in range(NCORE)], axis=0)
    return np.ascontiguousarray(out.astype(np.float32))
